# revision 17
# baseline (speedup 1.0000x reference)
"""CRF Viterbi decode kernel for Trainium2 (8 NeuronCores, data-parallel batch).

Per core (128 sequences, batch on partitions):

  Phase A (overlapped): X arrives host-pretransposed [D, S, BC]; DMA slabs
    feed PE matmul lhsT directly. 4-step chunks -> PSUM -> one ACT copy per
    chunk into e_store, streamed front/back interleaved ~24 chunks ahead of
    the scan (xstage/PSUM pool depth is the real prefetch limit). T and T^T
    are both host-provided so boot needs no on-chip transpose/gather.

  Scan: 511 pairs, TWO custom DVE ops each (VITSTEP_ANT, pure-COUNT 4-state
    FSM; stream = 26 pages x 27 elems; j<26: acc = max(acc, T+in1[j]);
    j=26 adds the emission e[page] without consuming src1). Everything is
    kept CONTIGUOUS on the DVE (measured: strided src/dst costs ~1.8x):
    in0 = T+e buffer (contiguous), out = page-major scratch (contiguous),
    in1 = stride-1 broadcast of the previous step's 26 values:
      fwd: in1 = d3[k]      (delta, ACT-persisted from scratch col 26)
      bwd: in1 = c_pp[k-1]  (c = beta+e, ACT-persisted from col 26)
    Each persist runs inside the OTHER direction's op window, so the DVE
    chain never stalls. Interleaved independent ops measure 810 ns each.
    ACT ops are ~390 ns, so the rest is batched: e-slot scatters 2 pairs
    per op (block-reversed for bwd so strides stay positive), beta
    persists 4 pairs per op (negative-stride out). GPSIMD parks
    gamma = delta + beta into d_store in 4-step blocks.
    beta storage: t >= 256 in b3h (half store), t < 256 in a 16-slot
    rolling window (consumed within ~4 pairs by gamma parking).

  Tail: onehot = (gamma >= rowmax) via DVE reduce_max + is_ge in-place in
    d_store, in 32-step pieces as soon as both directions have covered
    them (8-step edge pieces post-scan); DMA out overlaps the scan. (A global V*-threshold compare was
    measured unsafe: f32 noise +-0.02 overlaps the margin distribution.)
"""

import numpy as np

B, S, D, L = 1024, 512, 128, 26
NCORES = 8
BC = B // NCORES
HALF = S // 2
NP = S - 1
NSC = 4         # scratch slots per direction
NB2 = 2         # in0 pair-slot buffers per direction (2 pairs each)
SCHUNK = 4
LA = 8          # phase-A lookahead (chunk-pairs)
TCH = 32        # tail sub-chunk (steps)
WB = 16         # b_win slots

_BUILD_CACHE = {}


def _np_vitstep(in0, in1, c0, c1, c2):
    i0 = np.asarray(in0, np.float32).reshape(in0.shape[0], -1, 27)
    T = i0[..., :26]
    e = i0[..., 26]
    i1 = np.asarray(in1, np.float32).reshape(in1.shape[0], -1, 26)[..., :26]
    s = T + i1
    r = np.maximum.accumulate(s, axis=-1)
    out = np.empty_like(i0)
    out[..., :26] = r
    out[..., 26] = r[..., 25] + e
    return out.reshape(in0.shape)


def register_vitstep():
    from concourse import dve_spec as Dv
    from concourse import dve_ops as DO
    from concourse.dve_spec import Spec, Src0, Src1, scan, AluOp
    from concourse.dve_uop import DveOpSpec, AluInp, Trigger

    for op in DO.OPS:
        if op.name == "VITSTEP_ANT":
            return op

    SRC_DONE = Trigger.SRC_TENSOR_DONE
    CNT = Trigger.COUNT
    NONE = Trigger.NONE

    def _lower_vitstep(spec, ver):
        Dv._validate_body(spec, ver)
        spec2 = Dv._hoist_stream_invariant_ops(spec)
        scans = Dv._collect(spec2.body, Dv.Scan)
        p = Dv._build_placement(spec2, scans, Dv.N_STAGES[ver], Dv.N_LANES[ver])
        base_states = list(Dv._build_state_machine(spec2, scans, [], p))
        assert len(base_states) == 2, base_states
        consume = base_states[1].consume
        assert consume == (True, True)
        sc = scans[0]
        init = Dv._scan_init(sc)
        d = p.node_stage[sc]
        reset_ov = {d: Dv._Stage(sc.op, init, sc.expr)}
        adde_ov = {
            0: Dv._Stage(AluOp.BYPASS, Src0),
            d: Dv._Stage(AluOp.ADD, AluInp.CURR_ALU_OUT, Dv.PREV),
        }
        mk = Dv._State
        states = [
            mk(placement=p, consume=consume, overrides=reset_ov,
               trigger=(SRC_DONE, CNT, NONE), next=(0, 1, 0), repeat=1),
            mk(placement=p, consume=consume,
               trigger=(SRC_DONE, CNT, NONE), next=(0, 2, 0), repeat=25),
            mk(placement=p, consume=(True, False), overrides=adde_ov,
               trigger=(SRC_DONE, CNT, NONE), next=(0, 3, 0), repeat=1),
            mk(placement=p, consume=consume, overrides=reset_ov,
               trigger=(SRC_DONE, CNT, NONE), next=(0, 1, 0), repeat=1),
        ]
        out = [Dv._assemble(s) for s in states]
        for u in out:
            u.validate(ver)
        return out

    class VitDveOp(DO.DveOp):
        def compile(self, ver):
            key = (self.name, ver)
            if (r := DO._COMPILE_CACHE.get(key)) is not None:
                return r
            result = DveOpSpec(
                name=self.name,
                opcode=DO.get_dve_sub_opcode(self.name),
                uops=_lower_vitstep(self.spec, ver),
                rd1_en=DO.has_src1(self.spec),
            )
            DO._COMPILE_CACHE[key] = result
            return result

    spec = Spec(body=scan(AluOp.MAX, Src0 + Src1), reference=_np_vitstep)
    op = VitDveOp("VITSTEP_ANT", spec, subdim=False, uops_sha={})
    DO.OPS.append(op)
    DO._SUB_OPCODE_FOR_NAME[op.name] = DO._CUSTOM_DVE_ROW_BASE + len(DO.OPS) - 1
    DO.CUSTOM_DVE_SPECS[op.name] = spec
    assert DO._SUB_OPCODE_FOR_NAME[op.name] < 0x20
    return op


def _build(s_len):
    import concourse.bass as bass
    import concourse.bacc as bacc
    import concourse.tile as tile
    import concourse.mybir as mybir

    vit = register_vitstep()
    alu = mybir.AluOpType
    f32 = mybir.dt.float32

    half = s_len // 2
    np_ = s_len - 1
    nch = s_len // SCHUNK

    nc = bacc.Bacc("TRN2", target_bir_lowering=False, debug=False)
    Xh = nc.dram_tensor("XT", (D, s_len, BC), f32, kind="ExternalInput")
    Wh = nc.dram_tensor("W", (D, L), f32, kind="ExternalInput")
    Th = nc.dram_tensor("T", (L, L), f32, kind="ExternalInput")
    Tth = nc.dram_tensor("TT", (L, L), f32, kind="ExternalInput")
    Oh = nc.dram_tensor("OUT", (BC, s_len, L), f32, kind="ExternalOutput")

    def ap_at(t, offset_elems, dims):
        a = t[:]
        return bass.AP(tensor=a.tensor, offset=a.offset + offset_elems,
                       ap=[list(a.ap[0])] + dims)

    chunk_order = []
    for c in range(nch // 2):
        chunk_order += [c, nch - 1 - c]
    if nch % 2:
        chunk_order.append(nch // 2)

    # gamma block schedule: pair -> list of (t0, t1, src) to park
    #   src: ("b3h", off) | ("bwin", slot)
    gamma_at = {}
    for k in range(259, np_ - 3, 4):          # fwd side: ts k-3..k  (256..507)
        gamma_at.setdefault(k, []).append((k - 3, k + 1, "b3h"))
    for t0 in range(4, half, 4):              # bwd side: ts t0..t0+3 (4..255)
        gamma_at.setdefault(514 - t0, []).append((t0, t0 + 4, "bwin"))
    post_gamma = [(0, 4, "bwin"), (s_len - 4, s_len - 1, "b3h")]

    # tail schedule: pieces (t0, tlen); 8-step edges go post-scan
    pieces = [(0, 8), (8, 24)] + [(t, 32) for t in range(32, s_len - 32, 32)] \
        + [(s_len - 32, 24), (s_len - 8, 8)]
    tail_at = {}
    post_tail = []
    for (t0, tl) in pieces:
        t_hi = t0 + tl - 1
        if t0 < 4 or t_hi >= s_len - 4:
            post_tail.append((t0, tl))
            continue
        r = (515 - t0) if t_hi < half else (t_hi + 4)
        if r <= np_ - 2:
            tail_at.setdefault(r, []).append((t0, tl))
        else:
            post_tail.append((t0, tl))

    with tile.TileContext(nc) as tc:
        with (
            tc.tile_pool(name="singles", bufs=1) as singles,
            tc.tile_pool(name="xstage", bufs=6) as xstage_p,
            tc.tile_pool(name="ps_e", bufs=8, space="PSUM") as ps_e,
        ):
            e_store = singles.tile([BC, s_len * L], f32)
            d_store = singles.tile([BC, s_len * L], f32)
            b3h = singles.tile([BC, half * L], f32)     # beta, t >= half
            b_win = singles.tile([BC, WB * L], f32)     # beta, t < half
            t_cat = singles.tile([BC, 2, L, L], f32)
            # in0 pair-slot bufs: [dir-buf][2 pairs][26 pages][27]
            in0f = singles.tile([BC, NB2, 2, L * 27], f32)
            in0b = singles.tile([BC, NB2, 2, L * 27], f32)
            scrf = singles.tile([BC, NSC, 704], f32)
            scrb = singles.tile([BC, NSC, 704], f32)
            c_pp = singles.tile([BC, 2, L], f32)
            gm = singles.tile([BC, TCH], f32)
            w_sb = singles.tile([D, L], f32)
            e3 = e_store.rearrange("p (s l) -> p s l", l=L)

            nc.sync.dma_start(w_sb[:], Wh[:])

            def emit_chunk(cidx):
                c0 = cidx * SCHUNK
                xs = xstage_p.tile([D, SCHUNK, BC], f32)
                nc.sync.dma_start(xs[:], Xh[:, c0:c0 + SCHUNK, :])
                ep = ps_e.tile([BC, SCHUNK, L], f32)
                for si in range(SCHUNK):
                    nc.tensor.matmul(ep[:, si, :], lhsT=xs[:, si, :],
                                     rhs=w_sb[:], start=True, stop=True)
                nc.scalar.copy(e3[:, c0:c0 + SCHUNK, :], ep[:])

            emitted = 0

            def ensure_chunks(n):
                nonlocal emitted
                while emitted < min(n, nch):
                    emit_chunk(chunk_order[emitted])
                    emitted += 1

            t_ap = Th[:]
            nc.sync.dma_start(
                t_cat[:, 1, :, :].rearrange("p a b -> p (a b)"),
                bass.AP(tensor=t_ap.tensor, offset=t_ap.offset,
                        ap=[[0, BC], [1, L * L]]),
            )
            tt_ap = Tth[:]
            nc.sync.dma_start(
                t_cat[:, 0, :, :].rearrange("p a b -> p (a b)"),
                bass.AP(tensor=tt_ap.tensor, offset=tt_ap.offset,
                        ap=[[0, BC], [1, L * L]]),
            )
            ensure_chunks(6)

            for b in range(NB2):
                for blk in range(2):
                    off = (b * 2 + blk) * (L * 27)
                    nc.scalar.copy(
                        ap_at(in0f, off, [[27, L], [1, L]]), t_cat[:, 0])
                    nc.scalar.copy(
                        ap_at(in0b, off, [[27, L], [1, L]]), t_cat[:, 1])

            d3 = d_store.rearrange("p (s l) -> p s l", l=L)


            # scatter e-slots for the pair group {p, p+1} (p even)
            def scatter_group(p):
                buf = (p // 2) % NB2
                if p == np_ - 1:  # last group has a single pair (510)
                    nc.scalar.copy(
                        ap_at(in0f, (buf * 2) * (L * 27) + 26, [[27, L]]),
                        e3[:, p + 1, :])
                    nc.scalar.copy(
                        ap_at(in0b, (buf * 2 + 1) * (L * 27) + 26, [[27, L]]),
                        e3[:, np_ - 1 - p, :])
                    return
                # fwd: block j holds pair p+j -> e_{p+1+j}; ascending
                nc.scalar.copy(
                    ap_at(in0f, (buf * 2) * (L * 27) + 26,
                          [[L * 27, 2], [27, L]]),
                    e3[:, p + 1:p + 3, :])
                # bwd: block j holds pair p+1-j -> block0=e_{509-p},
                # block1=e_{510-p}; in ascending {509-p, 510-p}
                nc.scalar.copy(
                    ap_at(in0b, (buf * 2) * (L * 27) + 26,
                          [[L * 27, 2], [27, L]]),
                    e3[:, np_ - 2 - p:np_ - p, :])

            # boot: d3[0] = e_0; scatter pair groups {0,1} and {2,3}
            nc.scalar.copy(d3[:, 0, :], e3[:, 0, :])
            scatter_group(0)
            scatter_group(2)

            def emit_tail(piece):
                c0, tl = piece
                gsrc = d3[:, c0:c0 + tl, :]
                nc.vector.reduce_max(gm[:, :tl], gsrc,
                                     axis=mybir.AxisListType.X)
                gm_bc = (gm[:, :tl].rearrange("p (t o) -> p t o", o=1)
                         .broadcast_to((BC, tl, L)))
                nc.vector.tensor_tensor(gsrc, gsrc, gm_bc, op=alu.is_ge)
                nc.sync.dma_start(
                    Oh[:, c0:c0 + tl, :].rearrange("p s l -> p (s l)"),
                    d_store[:, c0 * L:(c0 + tl) * L])

            def bpersist_to(bt, n, sf0):
                """ACT: b[bt], b[bt-1], .., b[bt-n+1] <- scrb slots sf0..sf0+n-1
                col 25 (bt descending as slot ascends)."""
                src = ap_at(scrb, sf0 * 704 + 25, [[704, n], [27, L]]) if n > 1 \
                    else ap_at(scrb, sf0 * 704 + 25, [[27, L]])
                if bt >= half:
                    assert bt - n + 1 >= half
                    dst = ap_at(b3h, (bt - half) * L, [[-L, n], [1, L]]) \
                        if n > 1 else ap_at(b3h, (bt - half) * L, [[1, L]])
                    nc.scalar.copy(dst, src)
                else:
                    # b_win slots bt%WB descending; split at window wrap
                    done = 0
                    while done < n:
                        b0 = bt - done
                        run = min(n - done, b0 % WB + 1)
                        s = ap_at(scrb, (sf0 + done) * 704 + 25,
                                  [[704, run], [27, L]]) if run > 1 else \
                            ap_at(scrb, (sf0 + done) * 704 + 25, [[27, L]])
                        dd = ap_at(b_win, (b0 % WB) * L, [[-L, run], [1, L]]) \
                            if run > 1 else ap_at(b_win, (b0 % WB) * L, [[1, L]])
                        nc.scalar.copy(dd, s)
                        done += run

            # ---- scan pairs ----
            for k in range(np_):
                ft, bt = k + 1, s_len - 2 - k
                buf, blk, sf = (k // 2) % NB2, k % 2, k % NSC
                # DVE ops
                if k == 0:
                    in1f = ap_at(e_store, 0, [[0, L], [1, L]])
                    in1b = ap_at(e_store, (s_len - 1) * L, [[0, L], [1, L]])
                else:
                    in1f = ap_at(d_store, k * L, [[0, L], [1, L]])
                    in1b = ap_at(c_pp, ((k - 1) % 2) * L, [[0, L], [1, L]])
                nc.vector._custom_dve(
                    vit, out=ap_at(scrf, sf * 704, [[27, L], [1, 27]]),
                    in0=ap_at(in0f, (buf * 2 + blk) * (L * 27),
                              [[27, L], [1, 27]]),
                    in1=in1f)
                nc.vector._custom_dve(
                    vit, out=ap_at(scrb, sf * 704, [[27, L], [1, 27]]),
                    in0=ap_at(in0b, (buf * 2 + (1 - blk)) * (L * 27),
                              [[27, L], [1, 27]]),
                    in1=in1b)
                # ACT chain persists (each hidden under the other dir's op)
                nc.scalar.copy(d3[:, ft, :], ap_at(scrf, sf * 704 + 26,
                                                   [[27, L]]))
                nc.scalar.copy(ap_at(c_pp, (k % 2) * L, [[1, L]]),
                               ap_at(scrb, sf * 704 + 26, [[27, L]]))
                # batched beta persist: at k%4==3, pairs k-3..k (slots 0..3)
                if k % 4 == 3:
                    if bt + 3 >= half and bt < half:
                        nh = bt + 4 - half      # in b3h
                        bpersist_to(bt + 3, nh, 0)
                        bpersist_to(bt + 3 - nh, 4 - nh, nh)
                    else:
                        bpersist_to(bt + 3, 4, 0)
                elif k >= np_ - 3:              # tail pairs 508..510: singles
                    bpersist_to(bt, 1, sf)
                # scatters for pair group {k+3, k+4} at odd k
                if k % 2 == 1 and k + 3 <= np_ - 1:
                    scatter_group(k + 3)
                # GPSIMD gamma parking
                for (t0, t1, src) in gamma_at.get(k, ()):
                    if src == "b3h":
                        bsrc = ap_at(b3h, (t0 - half) * L, [[1, (t1 - t0) * L]])
                    else:
                        bsrc = ap_at(b_win, (t0 % WB) * L, [[1, (t1 - t0) * L]])
                    nc.gpsimd.tensor_tensor(
                        d_store[:, t0 * L:t1 * L],
                        d_store[:, t0 * L:t1 * L], bsrc, op=alu.add)
                for c in tail_at.get(k, ()):
                    emit_tail(c)
                if k % 2 == 0:
                    ensure_chunks(min(4 + k, k // 2 + 24))

            def park(t0, t1, src):
                if src == "b3h":
                    bsrc = ap_at(b3h, (t0 - half) * L, [[1, (t1 - t0) * L]])
                else:
                    bsrc = ap_at(b_win, (t0 % WB) * L, [[1, (t1 - t0) * L]])
                nc.gpsimd.tensor_tensor(
                    d_store[:, t0 * L:t1 * L],
                    d_store[:, t0 * L:t1 * L], bsrc, op=alu.add)

            park(s_len - 4, s_len - 1, "b3h")
            for piece in post_tail:
                if piece[0] >= half:
                    emit_tail(piece)
            park(0, 4, "bwin")
            for piece in post_tail:
                if piece[0] < half:
                    emit_tail(piece)

    nc.compile()
    return nc


def _get(s_len):
    if s_len not in _BUILD_CACHE:
        _BUILD_CACHE[s_len] = _build(s_len)
    return _BUILD_CACHE[s_len]


LAST_RESULT = None


def kernel(X, W, T):
    global LAST_RESULT
    from concourse.bass_utils import run_bass_kernel_spmd

    X = np.ascontiguousarray(X, dtype=np.float32)
    W = np.ascontiguousarray(W, dtype=np.float32)
    T = np.ascontiguousarray(T, dtype=np.float32)
    s_len = X.shape[1]
    nc = _get(s_len)
    in_maps = []
    for c in range(NCORES):
        xt = np.ascontiguousarray(
            X[c * BC:(c + 1) * BC].transpose(2, 1, 0))  # [D, S, BC]
        in_maps.append({"XT": xt, "W": W, "T": T,
                        "TT": np.ascontiguousarray(T.T)})
    res = run_bass_kernel_spmd(nc, in_maps, core_ids=list(range(NCORES)))
    LAST_RESULT = res
    return np.concatenate([r["OUT"] for r in res.results], axis=0)
